# revision 28
# baseline (speedup 1.0000x reference)
"""Trainium2 Bass kernel for a single-layer causal-attention decoder.

Key observation: VOCAB=5, so Q[i] = QV[x_i] and K[j] = KV[x_j] where
QV/KV are the 5 per-vocab projected rows. The whole [S, S] score matrix
is a gather from the 5x5 Gram table G = QV @ KV.T / sqrt(D):

    scores[i, j] = G[x_i, x_j]

With eg = exp(G) (no max-subtraction needed: |G| < ~6), the causal
softmax-attention collapses to per-vocab prefix counts:

    out[i] = (sum_v eg[x_i, v] * cnt_v(i) * VV[v]) /
             (sum_v eg[x_i, v] * cnt_v(i))

where cnt_v(i) = |{j <= i : x_j = v}|. Everything is O(S * V):

  - onehotT [5, S] (fp32r) from int32 x-broadcast vs iota (is_equal)
  - cntT [5, S] = inclusive prefix-sum of onehotT (DVE tensor_tensor_scan)
  - ET [5, S] = eg.T @ onehotT (fp32r matmuls; ET[v,i] = eg[x_i, v] exact)
  - WT [5, S] = ET * cntT (gpsimd, bf16 out)
  - per 128-row block: PSUM [128, 65] = WT_blk.T @ VV_aug (bf16, ones
    column gives the denominator), reciprocal on DVE, scale-multiply on
    ACT, batched DMA out per 512-row chunk.

Sharding: data-parallel over batch. B=8 -> 8 NeuronCores, one sequence
per core; weights replicated. No collectives.
"""

import numpy as np

import concourse.bass as bass
import concourse.mybir as mybir
import concourse.tile as tile
from concourse import bacc
from concourse.bass_utils import run_bass_kernel_spmd

F32 = mybir.dt.float32
F32R = mybir.dt.float32r
BF16 = mybir.dt.bfloat16
F16 = mybir.dt.float16
I32 = mybir.dt.int32
I16 = mybir.dt.int16

B = 8
S = 2048
D = 64
V = 5
P = 128
EC = 512  # chunk (PSUM bank free-dim limit for fp32)
N_CORES = 8
CBW = V + D + D + (D + 1)  # 5 + 64 + 64 + 65 = 198


def _body(tc, aps, S):
    nc = tc.nc
    x, cb, out = aps["x"], aps["cb"], aps["out"]
    ec = min(EC, S)
    NCH = S // ec        # chunks
    BPC = ec // P        # 128-row blocks per chunk
    Exp = mybir.ActivationFunctionType.Exp
    Copy = mybir.ActivationFunctionType.Copy

    from contextlib import ExitStack
    with ExitStack() as ctx:
        consts = ctx.enter_context(tc.tile_pool(name="consts", bufs=1))
        outp = ctx.enter_context(tc.tile_pool(name="outp", bufs=2))
        ps_small = ctx.enter_context(tc.tile_pool(name="ps_small", bufs=2, space="PSUM"))
        ps_et = ctx.enter_context(tc.tile_pool(name="ps_et", bufs=3, space="PSUM"))
        ps_o = ctx.enter_context(tc.tile_pool(name="ps_o", bufs=3, space="PSUM"))

        # ---- x (pre-broadcast on host) first: feeds the DVE critical chain ----
        xb = consts.tile([V, S], I16)
        nc.sync.dma_start(xb[:], x[None, :].to_broadcast((V, S)))
        io = consts.tile([V, 1], I32)
        nc.gpsimd.iota(io[:], pattern=[[0, 1]], base=0, channel_multiplier=1)
        io16 = consts.tile([V, 1], I16)
        nc.vector.tensor_copy(io16[:], io[:])

        # ---- constants in ----
        cb_sb = consts.tile([D + 1, CBW], F32)
        nc.sync.dma_start(cb_sb[:], cb[:])
        etT = cb_sb[:, 0:V]                          # [65, 5] emb_aug.T
        wqa = cb_sb[:, V : V + D]                    # [65, 64] wq_aug
        wka = cb_sb[:, V + D : V + 2 * D]            # [65, 64] wk_aug
        wva = cb_sb[:, V + 2 * D : V + 3 * D + 1]    # [65, 65] wv_aug + e_D col

        # ---- 5x5 score table: G = QV @ KV.T, eg = exp(G/8) ----
        # PSUM->SBUF staging copies ride on ACT so the DVE queue stays clear
        pqvt = ps_small.tile([D, V], F32, tag="small")
        nc.tensor.matmul(pqvt[:], lhsT=wqa, rhs=etT, start=True, stop=True)
        qvt_sb = consts.tile([D, V], F32)            # QVT[d, u] = QV[u, d]
        nc.scalar.copy(qvt_sb[:], pqvt[:])

        pkvt = ps_small.tile([D, V], F32, tag="small")
        nc.tensor.matmul(pkvt[:], lhsT=wka, rhs=etT, start=True, stop=True)
        kvt_sb = consts.tile([D, V], F32)
        nc.scalar.copy(kvt_sb[:], pkvt[:])

        pvv = ps_small.tile([V, D + 1], F32, tag="small")
        nc.tensor.matmul(pvv[:], lhsT=etT, rhs=wva, start=True, stop=True)
        vv_sb = consts.tile([V, D + 1], BF16)        # VV_aug, ones column at d=64
        nc.scalar.copy(vv_sb[:], pvv[:])

        pg = ps_small.tile([V, V], F32, tag="small")
        nc.tensor.matmul(pg[:], lhsT=qvt_sb[:], rhs=kvt_sb[:], start=True, stop=True)
        eg_sb = consts.tile([V, V], BF16)            # eg[u, v], lhsT for ET
        nc.scalar.activation(eg_sb[:], pg[:], Exp, scale=0.125)

        # ---- chunked pipeline over the sequence ----
        # One full-S one-hot up front, then per-chunk scan/WT/PV. The DVE
        # queue is interleaved (scan_{c+1} is emitted between wt_c and rc_c)
        # so DVE never idles waiting on the PE's PV matmuls.
        oh = consts.tile([V, S], BF16)   # 0/1 exact in bf16; feeds ET + scan
        cnt = consts.tile([V, S], F16)   # counts <= 2048, exact in fp16
        wt = consts.tile([V, S], BF16)
        rs_sb = outp.tile([P, S // P, D], F32, tag="rs")
        # out viewed as [chunk, 128, block, 64] so each chunk DMAs in one shot
        out_r = out.rearrange("(c b p) d -> c p b d", c=NCH, p=P)

        pets = [None] * NCH

        def one_hot(c0, c1):
            # one-hot: out = (xb == io16) bypass xb; the in1 slot is a dummy
            # 2-byte packed operand so the DVE 2x mode stays eligible
            sl = slice(c0 * ec, c1 * ec)
            nc.vector.scalar_tensor_tensor(
                oh[:, sl], xb[:, sl], io16[:, 0:1], xb[:, sl],
                op0=mybir.AluOpType.is_equal, op1=mybir.AluOpType.bypass,
            )

        def emit_et(c):
            pet = ps_et.tile([V, ec], F32, tag="et")
            nc.tensor.matmul(
                pet[:], lhsT=eg_sb[:], rhs=oh[:, c * ec : (c + 1) * ec],
                start=True, stop=True,
            )
            pets[c] = pet

        def scan(c):
            sl = slice(c * ec, (c + 1) * ec)
            # inclusive prefix count: state = (oh + state) bypass oh
            nc.vector.tensor_tensor_scan(
                cnt[:, sl], oh[:, sl], oh[:, sl],
                initial=0.0 if c == 0 else cnt[:, c * ec - 1 : c * ec],
                op0=mybir.AluOpType.add, op1=mybir.AluOpType.bypass,
            )

        pos = [None] * NCH
        rcs = [None] * NCH

        def emit_wt_pv(c):
            sl = slice(c * ec, (c + 1) * ec)
            nc.vector.tensor_tensor(
                wt[:, sl], pets[c][:], cnt[:, sl], mybir.AluOpType.mult,
            )
            po = ps_o.tile([P, BPC * (D + 1)], F32, tag="po")
            for b in range(BPC):
                blk = c * BPC + b
                nc.tensor.matmul(
                    po[:, b * (D + 1) : (b + 1) * (D + 1)],
                    lhsT=wt[:, blk * P : (blk + 1) * P], rhs=vv_sb[:],
                    start=True, stop=True,
                )
            pos[c] = po

        def emit_norm(c):
            po = pos[c]
            rc4 = outp.tile([P, BPC], F32, tag="rc")
            den = po[:].rearrange("p (b e) -> p b e", e=D + 1)[:, :, D : D + 1]
            nc.vector.reciprocal(rc4[:].unsqueeze(2), den)
            for b in range(BPC):
                blk = c * BPC + b
                nc.scalar.activation(
                    rs_sb[:, blk, :], po[:, b * (D + 1) : b * (D + 1) + D],
                    Copy, scale=rc4[:, b : b + 1],
                )
            nc.sync.dma_start(out_r[c], rs_sb[:, c * BPC : (c + 1) * BPC, :])

        def emit_norm_tail(c):
            # last chunk: normalize in one DVE op (broadcast-strided rc)
            po = pos[c]
            rc4 = outp.tile([P, BPC], F32, tag="rc")
            po4 = po[:].rearrange("p (b e) -> p b e", e=D + 1)
            nc.vector.reciprocal(rc4[:].unsqueeze(2), po4[:, :, D : D + 1])
            nc.vector.tensor_tensor(
                rs_sb[:, c * BPC : (c + 1) * BPC, :], po4[:, :, 0:D],
                rc4[:].unsqueeze(2).to_broadcast((P, BPC, D)),
                mybir.AluOpType.mult,
            )
            nc.sync.dma_start(out_r[c], rs_sb[:, c * BPC : (c + 1) * BPC, :])

        # DVE queue: ie0, scan0, ie1, scan1, ie23, wt0, scan2, rc0, wt1,
        # scan3, rc1, wt2, rc2, wt3, rc3a, norm3a, rc3b, norm3b
        one_hot(0, NCH)
        for c in range(min(3, NCH)):
            emit_et(c)
        scan(0)
        if NCH > 1:
            scan(1)
        for c in range(NCH):
            emit_wt_pv(c)
            if c + 3 < NCH:
                emit_et(c + 3)  # late: keeps ps_et at 3 live tiles
            if c + 2 < NCH:
                scan(c + 2)
            if c == NCH - 1:
                emit_norm_tail(c)
            else:
                emit_norm(c)


def build_nc(S=S, mode=None):
    nc = bacc.Bacc(trn_type="TRN2", target_bir_lowering=False, debug=False)
    aps = {}
    aps["x"] = nc.dram_tensor("x", [S], I16, kind="ExternalInput").ap()
    aps["cb"] = nc.dram_tensor("cb", [D + 1, CBW], F32, kind="ExternalInput").ap()
    aps["out"] = nc.dram_tensor("out", [S, D], F32, kind="ExternalOutput").ap()
    with tile.TileContext(nc) as tc:
        _body(tc, aps, S=S)
    nc.compile()
    return nc


def make_in_maps(x, emb_table, wq, bq, wk, bk, wv, bv, S=S, n_cores=N_CORES):
    x = np.asarray(x).astype(np.int16)
    emb_table = np.asarray(emb_table, dtype=np.float32)

    def aug(w, b):
        return np.vstack(
            [np.asarray(w, np.float32).T, np.asarray(b, np.float32)[None, :]]
        )  # [D+1, D]

    cbuf = np.zeros((D + 1, CBW), np.float32)
    cbuf[:, 0:V] = np.vstack([emb_table.T, np.ones((1, V), np.float32)])
    cbuf[:, V : V + D] = aug(wq, bq)
    cbuf[:, V + D : V + 2 * D] = aug(wk, bk)
    cbuf[:, V + 2 * D : V + 3 * D] = aug(wv, bv)
    cbuf[D, V + 3 * D] = 1.0  # e_D column of wv_aug -> ones column of VV_aug
    cbuf = np.ascontiguousarray(cbuf)

    return [
        dict(x=np.ascontiguousarray(x[c, :S]), cb=cbuf)
        for c in range(n_cores)
    ]


_NC_CACHE = {}

MODE = "bf16"  # W@VV runs bf16; everything upstream is fp32/fp32r-exact


def _get_nc(S=S, mode=None):
    key = S
    if key not in _NC_CACHE:
        _NC_CACHE[key] = build_nc(S=S)
    return _NC_CACHE[key]


def run(inputs, trace=False, **kw):
    in_maps = make_in_maps(**inputs)
    nc = _get_nc()
    res = run_bass_kernel_spmd(nc, in_maps, core_ids=list(range(N_CORES)), trace=trace, **kw)
    out = np.stack([res.results[c]["out"] for c in range(N_CORES)])
    return out, res


def kernel(x, emb_table, wq, bq, wk, bk, wv, bv):
    out, _ = run(dict(x=x, emb_table=emb_table, wq=wq, bq=bq, wk=wk, bk=bk,
                      wv=wv, bv=bv))
    return out


# revision 30
# speedup vs baseline: 1.0248x; 1.0248x over previous
"""Trainium2 Bass kernel for a single-layer causal-attention decoder.

Key observation: VOCAB=5, so Q[i] = QV[x_i] and K[j] = KV[x_j] where
QV/KV are the 5 per-vocab projected rows. The whole [S, S] score matrix
is a gather from the 5x5 Gram table G = QV @ KV.T / sqrt(D):

    scores[i, j] = G[x_i, x_j]

With eg = exp(G) (no max-subtraction needed: |G| < ~6), the causal
softmax-attention collapses to per-vocab prefix counts:

    out[i] = (sum_v eg[x_i, v] * cnt_v(i) * VV[v]) /
             (sum_v eg[x_i, v] * cnt_v(i))

where cnt_v(i) = |{j <= i : x_j = v}|. Everything is O(S * V):

  - onehotT [5, S] (bf16) from int16 x-broadcast vs iota (DVE is_equal)
  - cntT [5, S] = inclusive prefix-sum of onehotT (DVE tensor_tensor_scan,
    fp16 out - counts <= 2048 are exact)
  - ET [5, S] = eg.T @ onehotT (bf16 PE matmuls; ET[v,i] = eg[x_i, v])
  - WT [5, S] = ET(PSUM) * cntT (DVE, bf16 out)
  - per 512 chunk: four PSUM [128, 65] = WT_blk.T @ VV_aug matmuls (bf16,
    ones column gives the denominator) into one bank, one strided
    reciprocal, per-block scale-multiply on ACT (tail chunk: one
    broadcast-strided DVE multiply), one DMA per chunk.
  Engine placement: DVE owns the serial chain (one-hot/scan/WT/recip),
  ACT stages tables + output scaling, PE does all matmuls; the fixed
  Bacc preamble/epilogue (barriers + sem-range reset) costs ~10us of the
  measured window.

Sharding: data-parallel over batch. B=8 -> 8 NeuronCores, one sequence
per core; weights replicated. No collectives.
"""

import numpy as np

import concourse.bass as bass
import concourse.mybir as mybir
import concourse.tile as tile
from concourse import bacc
from concourse.bass_utils import run_bass_kernel_spmd

F32 = mybir.dt.float32
F32R = mybir.dt.float32r
BF16 = mybir.dt.bfloat16
F16 = mybir.dt.float16
I32 = mybir.dt.int32
I16 = mybir.dt.int16

B = 8
S = 2048
D = 64
V = 5
P = 128
EC = 512  # chunk (PSUM bank free-dim limit for fp32)
N_CORES = 8
CBW = V + D + D + (D + 1)  # 5 + 64 + 64 + 65 = 198


def _body(tc, aps, S):
    nc = tc.nc
    x, cb, out = aps["x"], aps["cb"], aps["out"]
    ec = min(EC, S)
    NCH = S // ec        # chunks
    BPC = ec // P        # 128-row blocks per chunk
    Exp = mybir.ActivationFunctionType.Exp
    Copy = mybir.ActivationFunctionType.Copy

    from contextlib import ExitStack
    with ExitStack() as ctx:
        consts = ctx.enter_context(tc.tile_pool(name="consts", bufs=1))
        outp = ctx.enter_context(tc.tile_pool(name="outp", bufs=2))
        ps_small = ctx.enter_context(tc.tile_pool(name="ps_small", bufs=2, space="PSUM"))
        ps_et = ctx.enter_context(tc.tile_pool(name="ps_et", bufs=3, space="PSUM"))
        ps_o = ctx.enter_context(tc.tile_pool(name="ps_o", bufs=3, space="PSUM"))

        # ---- x (pre-broadcast on host) first: feeds the DVE critical chain ----
        xb = consts.tile([V, S], F16)
        nc.sync.dma_start(xb[:], x[None, :].to_broadcast((V, S)))
        io = consts.tile([V, 1], I32)
        nc.gpsimd.iota(io[:], pattern=[[0, 1]], base=0, channel_multiplier=1)
        io16 = consts.tile([V, 1], F16)
        nc.vector.tensor_copy(io16[:], io[:])

        # ---- constants in ----
        cb_sb = consts.tile([D + 1, CBW], F32)
        nc.sync.dma_start(cb_sb[:], cb[:])
        etT = cb_sb[:, 0:V]                          # [65, 5] emb_aug.T
        wqa = cb_sb[:, V : V + D]                    # [65, 64] wq_aug
        wka = cb_sb[:, V + D : V + 2 * D]            # [65, 64] wk_aug
        wva = cb_sb[:, V + 2 * D : V + 3 * D + 1]    # [65, 65] wv_aug + e_D col

        # ---- 5x5 score table: G = QV @ KV.T, eg = exp(G/8) ----
        # PSUM->SBUF staging copies ride on ACT so the DVE queue stays clear
        pqvt = ps_small.tile([D, V], F32, tag="small")
        nc.tensor.matmul(pqvt[:], lhsT=wqa, rhs=etT, start=True, stop=True)
        qvt_sb = consts.tile([D, V], F32)            # QVT[d, u] = QV[u, d]
        nc.scalar.copy(qvt_sb[:], pqvt[:])

        pkvt = ps_small.tile([D, V], F32, tag="small")
        nc.tensor.matmul(pkvt[:], lhsT=wka, rhs=etT, start=True, stop=True)
        kvt_sb = consts.tile([D, V], F32)
        nc.scalar.copy(kvt_sb[:], pkvt[:])

        pvv = ps_small.tile([V, D + 1], F32, tag="small")
        nc.tensor.matmul(pvv[:], lhsT=etT, rhs=wva, start=True, stop=True)
        vv_sb = consts.tile([V, D + 1], BF16)        # VV_aug, ones column at d=64
        nc.scalar.copy(vv_sb[:], pvv[:])

        pg = ps_small.tile([V, V], F32, tag="small")
        nc.tensor.matmul(pg[:], lhsT=qvt_sb[:], rhs=kvt_sb[:], start=True, stop=True)
        eg_sb = consts.tile([V, V], BF16)            # eg[u, v], lhsT for ET
        nc.scalar.activation(eg_sb[:], pg[:], Exp, scale=0.125)

        # ---- chunked pipeline over the sequence ----
        # One full-S one-hot up front, then per-chunk scan/WT/PV. The DVE
        # queue is interleaved (scan_{c+1} is emitted between wt_c and rc_c)
        # so DVE never idles waiting on the PE's PV matmuls.
        oh = consts.tile([V, S], BF16)   # 0/1 exact in bf16; feeds ET + scan
        cnt = consts.tile([V, S], F16)   # counts <= 2048, exact in fp16
        wt = consts.tile([V, S], BF16)
        rs_sb = outp.tile([P, S // P, D], F32, tag="rs")
        # out viewed as [chunk, 128, block, 64] so each chunk DMAs in one shot
        out_r = out.rearrange("(c b p) d -> c p b d", c=NCH, p=P)

        pets = [None] * NCH

        def one_hot(c0, c1):
            # one-hot: out = (xb == io16) bypass xb; the in1 slot is a dummy
            # 2-byte packed operand so the DVE 2x mode stays eligible
            sl = slice(c0 * ec, c1 * ec)
            nc.vector.scalar_tensor_tensor(
                oh[:, sl], xb[:, sl], io16[:, 0:1], xb[:, sl],
                op0=mybir.AluOpType.is_equal, op1=mybir.AluOpType.bypass,
            )

        def emit_et(c):
            pet = ps_et.tile([V, ec], F32, tag="et")
            nc.tensor.matmul(
                pet[:], lhsT=eg_sb[:], rhs=oh[:, c * ec : (c + 1) * ec],
                start=True, stop=True,
            )
            pets[c] = pet

        def scan(c):
            sl = slice(c * ec, (c + 1) * ec)
            # inclusive prefix count: state = (oh + state) bypass oh
            nc.vector.tensor_tensor_scan(
                cnt[:, sl], oh[:, sl], oh[:, sl],
                initial=0.0 if c == 0 else cnt[:, c * ec - 1 : c * ec],
                op0=mybir.AluOpType.add, op1=mybir.AluOpType.bypass,
            )

        pos = [None] * NCH
        rcs = [None] * NCH

        def emit_wt_pv(c):
            sl = slice(c * ec, (c + 1) * ec)
            nc.vector.tensor_tensor(
                wt[:, sl], pets[c][:], cnt[:, sl], mybir.AluOpType.mult,
            )
            po = ps_o.tile([P, BPC * (D + 1)], F32, tag="po")
            for b in range(BPC):
                blk = c * BPC + b
                nc.tensor.matmul(
                    po[:, b * (D + 1) : (b + 1) * (D + 1)],
                    lhsT=wt[:, blk * P : (blk + 1) * P], rhs=vv_sb[:],
                    start=True, stop=True,
                )
            pos[c] = po

        def emit_norm(c):
            po = pos[c]
            rc4 = outp.tile([P, BPC], F32, tag="rc")
            den = po[:].rearrange("p (b e) -> p b e", e=D + 1)[:, :, D : D + 1]
            nc.vector.reciprocal(rc4[:].unsqueeze(2), den)
            for b in range(BPC):
                blk = c * BPC + b
                nc.scalar.activation(
                    rs_sb[:, blk, :], po[:, b * (D + 1) : b * (D + 1) + D],
                    Copy, scale=rc4[:, b : b + 1],
                )
            nc.sync.dma_start(out_r[c], rs_sb[:, c * BPC : (c + 1) * BPC, :])

        def emit_norm_tail(c):
            # last chunk: normalize in one DVE op (broadcast-strided rc)
            po = pos[c]
            rc4 = outp.tile([P, BPC], F32, tag="rc")
            po4 = po[:].rearrange("p (b e) -> p b e", e=D + 1)
            nc.vector.reciprocal(rc4[:].unsqueeze(2), po4[:, :, D : D + 1])
            nc.vector.tensor_tensor(
                rs_sb[:, c * BPC : (c + 1) * BPC, :], po4[:, :, 0:D],
                rc4[:].unsqueeze(2).to_broadcast((P, BPC, D)),
                mybir.AluOpType.mult,
            )
            nc.sync.dma_start(out_r[c], rs_sb[:, c * BPC : (c + 1) * BPC, :])

        # DVE queue: ie0, scan0, ie1, scan1, ie23, wt0, scan2, rc0, wt1,
        # scan3, rc1, wt2, rc2, wt3, rc3a, norm3a, rc3b, norm3b
        one_hot(0, NCH)
        for c in range(min(3, NCH)):
            emit_et(c)
        scan(0)
        if NCH > 1:
            scan(1)
        for c in range(NCH):
            emit_wt_pv(c)
            if c + 3 < NCH:
                emit_et(c + 3)  # late: keeps ps_et at 3 live tiles
            if c + 2 < NCH:
                scan(c + 2)
            if c == NCH - 1:
                emit_norm_tail(c)
            else:
                emit_norm(c)


def build_nc(S=S, mode=None):
    nc = bacc.Bacc(trn_type="TRN2", target_bir_lowering=False, debug=False)
    aps = {}
    aps["x"] = nc.dram_tensor("x", [S], F16, kind="ExternalInput").ap()
    aps["cb"] = nc.dram_tensor("cb", [D + 1, CBW], F32, kind="ExternalInput").ap()
    aps["out"] = nc.dram_tensor("out", [S, D], F32, kind="ExternalOutput").ap()
    with tile.TileContext(nc) as tc:
        _body(tc, aps, S=S)
    nc.compile()
    return nc


def make_in_maps(x, emb_table, wq, bq, wk, bk, wv, bv, S=S, n_cores=N_CORES):
    x = np.asarray(x).astype(np.float16)
    emb_table = np.asarray(emb_table, dtype=np.float32)

    def aug(w, b):
        return np.vstack(
            [np.asarray(w, np.float32).T, np.asarray(b, np.float32)[None, :]]
        )  # [D+1, D]

    cbuf = np.zeros((D + 1, CBW), np.float32)
    cbuf[:, 0:V] = np.vstack([emb_table.T, np.ones((1, V), np.float32)])
    cbuf[:, V : V + D] = aug(wq, bq)
    cbuf[:, V + D : V + 2 * D] = aug(wk, bk)
    cbuf[:, V + 2 * D : V + 3 * D] = aug(wv, bv)
    cbuf[D, V + 3 * D] = 1.0  # e_D column of wv_aug -> ones column of VV_aug
    cbuf = np.ascontiguousarray(cbuf)

    return [
        dict(x=np.ascontiguousarray(x[c, :S]), cb=cbuf)
        for c in range(n_cores)
    ]


_NC_CACHE = {}

MODE = "bf16"  # W@VV runs bf16; everything upstream is fp32/fp32r-exact


def _get_nc(S=S, mode=None):
    key = S
    if key not in _NC_CACHE:
        _NC_CACHE[key] = build_nc(S=S)
    return _NC_CACHE[key]


def run(inputs, trace=False, **kw):
    in_maps = make_in_maps(**inputs)
    nc = _get_nc()
    res = run_bass_kernel_spmd(nc, in_maps, core_ids=list(range(N_CORES)), trace=trace, **kw)
    out = np.stack([res.results[c]["out"] for c in range(N_CORES)])
    return out, res


def kernel(x, emb_table, wq, bq, wk, bk, wv, bv):
    out, _ = run(dict(x=x, emb_table=emb_table, wq=wq, bq=bq, wk=wk, bk=bk,
                      wv=wv, bv=bv))
    return out
